# revision 1
# baseline (speedup 1.0000x reference)
"""Euclidean distance block (retrieval kNN) on 8 TRN2 NeuronCores.

dist[b, s, p] = sqrt(sum_c (x1[b, c, p] - x2[b, s, c, p])^2)   p = spatial (h*w)
out[b] = dist[b].reshape(S * h * w)

Sharding: data-parallel over batch B=32 -> 4 batches per core, no comms.

Per-core kernel layout: SBUF partitions carry (support_pair, channel) = 2*64 =
128; the free axis carries spatial. A big tile covers 8 supports as
[128, 4, 1764], streamed as four fully-contiguous 902 KB pair-DMAs (f32 HBM
-> bf16 SBUF cast on the SWDGE ring; per-pair DMAs give 4x finer completion
sems so compute starts on the first pair). Compute chain per tile:
  DVE subtract in bf16 (2x mode), in place
  Square -> bf16: 3 slices on ACT, 1 on DVE (engine cadence balance)
  PE matmul against [128, 25] one-hot pair masks, accumulating per-support
    sums over C into a [25, 441] PSUM tile per spatial quarter
  ACT Sqrt PSUM -> SBUF f32, one contiguous 176 KB store per batch on the
    Scalar HWDGE ring (which never blocks loads).

DMA ring assignment matters: HWDGE rings execute FIFO per issuing engine, so
a store waiting on compute would stall every load queued behind it. Loads
(with cast) go on the GpSimd SWDGE ring, stores on Scalar, and the sync ring
only carries the mask load and the x1 partition-duplicate copy.
"""

import numpy as np

B, S, C, H, W = 32, 25, 64, 42, 42
HW = H * W            # 1764
NCORES = 8
BL = B // NCORES      # 4 batches per core
NSO = 4               # support pairs per big tile (8 supports)
NBIG = 3              # big tiles per batch (24 supports), then 1 leftover
NQ = 4                # spatial quarters
QW = HW // NQ         # 441
NPAIR = 13            # 12 support pairs + 1 leftover single

BF16_SUB = True       # bf16 inputs to the subtract (2x DVE); False = f32
F32_RAMP = False      # stream b0's first tile in f32 on the sync ring

_cache = {}


def _build_nc():
    import concourse.bacc as bacc
    import concourse.mybir as mybir
    from concourse.tile import TileContext
    from concourse.bass import MemorySpace

    f32 = mybir.dt.float32
    bf16 = mybir.dt.bfloat16
    ldt = bf16 if BF16_SUB else f32
    Square = mybir.ActivationFunctionType.Square
    Sqrt = mybir.ActivationFunctionType.Sqrt
    sub = mybir.AluOpType.subtract

    # Square and Sqrt both live in the "sqrt_and_others" act-function set,
    # but the table-load chooser picks the first set containing each one,
    # alternating two ~2.7us table reloads per batch. Strip the two
    # functions from every other set (contents only — set ids are
    # positional) so one resident table serves the whole kernel.
    _orig_tables = bacc.get_activation_tables

    def _pinned_tables(arch):
        t = _orig_tables(arch)
        for name, fns in t.items():
            if name != "sqrt_and_others":
                fns.discard(Square)
                fns.discard(Sqrt)
        return t

    bacc.get_activation_tables = _pinned_tables
    nc = bacc.Bacc()
    x1 = nc.declare_dram_parameter("x1", [BL, C, HW], f32, isOutput=False)
    x2 = nc.declare_dram_parameter("x2", [BL, S, C, HW], f32, isOutput=False)
    mk = nc.declare_dram_parameter("mask", [NPAIR, 128, S], bf16, isOutput=False)
    out = nc.declare_dram_parameter("out", [BL, S * HW], f32, isOutput=True)

    # loads: cast f32->ldt needs SWDGE (gpsimd); plain copies can go anywhere
    load = nc.gpsimd if BF16_SUB else nc.sync

    with TileContext(nc) as tc:
        with (
            tc.tile_pool(name="x2p", bufs=6) as x2p,
            tc.tile_pool(name="sqp", bufs=3) as sqp,
            tc.tile_pool(name="x1p", bufs=1) as x1p,
            tc.tile_pool(name="outp", bufs=2) as outp,
            tc.tile_pool(name="cst", bufs=1) as cst,
            tc.tile_pool(name="x2fp", bufs=1) as x2fp,
            tc.tile_pool(name="ps", bufs=2, space=MemorySpace.PSUM) as psp,
        ):
            mt = cst.tile([128, NPAIR, S], bf16)
            nc.sync.dma_start(mt[:], mk.rearrange("g k m -> k g m"))

            # all of x1 once: [c, b, p] on partitions 0..63, then duplicate
            # onto 64..127 via SBUF->SBUF (no extra HBM traffic)
            x1all = x1p.tile([128, BL, HW], ldt)
            load.dma_start(x1all[0:64, :, :], x1.rearrange("b c p -> c b p"))
            nc.sync.dma_start(x1all[64:128, :, :], x1all[0:64, :, :])

            # The first gpsimd (SWDGE) DMA pays ~6us of Q7 library-load +
            # descriptor warmup before the first HBM byte moves. Stream the
            # first batch's first tile (and its x1) in f32 over the sync
            # HWDGE ring instead, so HBM traffic starts immediately.
            x1f = None
            if BF16_SUB and F32_RAMP:
                x1f = cst.tile([128, HW], f32, name="x1f")
                nc.sync.dma_start(x1f[0:64, :], x1[0])
                nc.sync.dma_start(x1f[64:128, :], x1[0])

            for b in range(BL):
                # b=0 starts on the warm sync ring in f32; leftover-first
                # ordering only for b>0 (for b=0 the leftover data arrives
                # late, after the gpsimd warmup)
                leftover_first = b > 0 or not (BF16_SUB and F32_RAMP)

                # leftover support 24: DMA early so it streams with big tiles
                x2l = x2p.tile([64, HW], ldt, tag="x2l")
                load.dma_start(x2l[:], x2[b, S - 1])

                pst = [
                    psp.tile([S, QW], f32, name=f"ps{q}", tag=f"ps{q}")
                    for q in range(NQ)
                ]

                def leftover_compute(b=b, x2l=x2l, pst=pst, first=True):
                    # leftover compute first keeps the end-of-batch tail short
                    nc.vector.tensor_tensor(x2l[:], x2l[:], x1all[0:64, b, :], sub)
                    sql = sqp.tile([64, HW], bf16, name="sql", tag="sql")
                    nc.scalar.activation(sql[:], x2l[:], Square)
                    for q in range(NQ):
                        nc.tensor.matmul(
                            pst[q][:, :],
                            mt[0:64, NPAIR - 1, :],
                            sql[:, q * QW : (q + 1) * QW],
                            start=first,
                            stop=not first,
                        )

                if leftover_first:
                    leftover_compute(first=True)

                for i in range(NBIG):
                    f32_tile = BF16_SUB and F32_RAMP and b == 0 and i == 0
                    if f32_tile:
                        x2t = x2fp.tile([128, NSO, HW], f32, name="x2tf", tag="x2tf")
                        x1s = x1f[:, :]
                        ring = nc.sync
                    else:
                        x2t = x2p.tile([128, NSO, HW], ldt, tag="x2t")
                        x1s = x1all[:, b, :]
                        ring = load
                    src = x2[b, 8 * i : 8 * i + 8].rearrange(
                        "(so si) c p -> (si c) so p", si=2
                    )
                    # per-pair DMAs: same streaming rate, but 4x finer
                    # completion sems -> subs start on the first 902KB
                    for so in range(NSO):
                        ring.dma_start(x2t[:, so, :], src[:, so, :])
                    # The very last tile's chain is the kernel tail: q-slice
                    # its compute so the final dependency chain is one
                    # 441-wide chunk instead of a whole 1764-wide slice.
                    last_tile = b == BL - 1 and i == NBIG - 1
                    ot = None
                    if last_tile:
                        ot = outp.tile([S, HW], f32, name="ot", tag="ot")
                    sq = sqp.tile([128, NSO, HW], bf16, tag="sq")
                    for so in range(NSO):
                        j = NSO * i + so
                        if not last_tile:
                            # in-place: x2t slice becomes diff
                            nc.vector.tensor_tensor(
                                x2t[:, so, :], x2t[:, so, :], x1s, sub
                            )
                            # squares split 3/1 across ACT and DVE to balance
                            # the per-tile engine cadence
                            if so < 3:
                                nc.scalar.activation(
                                    sq[:, so, :], x2t[:, so, :], Square
                                )
                            else:
                                nc.vector.tensor_tensor(
                                    sq[:, so, :],
                                    x2t[:, so, :],
                                    x2t[:, so, :],
                                    mybir.AluOpType.mult,
                                )
                            for q in range(NQ):
                                nc.tensor.matmul(
                                    pst[q][:, :],
                                    mt[:, j, :],
                                    sq[:, so, q * QW : (q + 1) * QW],
                                    start=(j == 0 and not leftover_first),
                                    stop=(j == NPAIR - 2 and leftover_first),
                                )
                        else:
                            for q in range(NQ):
                                qs = slice(q * QW, (q + 1) * QW)
                                nc.vector.tensor_tensor(
                                    x2t[:, so, qs], x2t[:, so, qs], x1s[:, qs], sub
                                )
                                if q % 2 == 0:
                                    nc.scalar.activation(
                                        sq[:, so, qs], x2t[:, so, qs], Square
                                    )
                                else:
                                    nc.vector.tensor_tensor(
                                        sq[:, so, qs],
                                        x2t[:, so, qs],
                                        x2t[:, so, qs],
                                        mybir.AluOpType.mult,
                                    )
                                nc.tensor.matmul(
                                    pst[q][:, :],
                                    mt[:, j, :],
                                    sq[:, so, qs],
                                    start=(j == 0 and not leftover_first),
                                    stop=(j == NPAIR - 2 and leftover_first),
                                )
                                if so == NSO - 1:
                                    # quarter q is complete: sqrt + store now
                                    nc.scalar.activation(
                                        ot[:, qs], pst[q][:], Sqrt
                                    )
                                    nc.scalar.dma_start(
                                        out[b].rearrange("(s p) -> s p", s=S)[:, qs],
                                        ot[:, qs],
                                    )

                if not leftover_first:
                    leftover_compute(first=False)

                if b < BL - 1:
                    ot = outp.tile([S, HW], f32, name="ot", tag="ot")
                    for q in range(NQ):
                        nc.scalar.activation(
                            ot[:, q * QW : (q + 1) * QW], pst[q][:], Sqrt
                        )
                    # store via the Scalar HWDGE ring: ACT reaches this only
                    # after its own sqrts, so the wait is pre-satisfied; a
                    # store on a load ring would stall loads queued behind it
                    nc.scalar.dma_start(out[b].rearrange("(s p) -> s p", s=S), ot[:])

    try:
        nc.finalize()
    finally:
        bacc.get_activation_tables = _orig_tables
    return nc


def get_nc():
    if "nc" not in _cache:
        _cache["nc"] = _build_nc()
    return _cache["nc"]


def make_mask() -> np.ndarray:
    # mask[j, k, m] = 1 iff partition k of pair-tile j feeds output support m.
    # Pair j < 12 covers supports (2j, 2j+1): k < 64 -> 2j, k >= 64 -> 2j+1.
    # Pair 12 is the leftover single support 24 on partitions 0..63.
    import ml_dtypes

    mask = np.zeros((NPAIR, 128, S), dtype=ml_dtypes.bfloat16)
    for j in range(NPAIR - 1):
        mask[j, 0:64, 2 * j] = 1.0
        mask[j, 64:128, 2 * j + 1] = 1.0
    mask[NPAIR - 1, 0:64, S - 1] = 1.0
    return mask


def make_in_maps(x1: np.ndarray, x2: np.ndarray) -> list[dict]:
    x1 = np.ascontiguousarray(np.asarray(x1, dtype=np.float32)).reshape(B, C, HW)
    x2 = np.ascontiguousarray(np.asarray(x2, dtype=np.float32)).reshape(B, S, C, HW)
    mask = make_mask()
    maps = []
    for i in range(NCORES):
        sl = slice(i * BL, (i + 1) * BL)
        maps.append({"x1": x1[sl], "x2": x2[sl], "mask": mask})
    return maps


def gather_out(results: list[dict]) -> np.ndarray:
    return np.concatenate([np.asarray(r["out"]) for r in results], axis=0).astype(
        np.float32, copy=False
    )


def kernel(x1, x2) -> np.ndarray:
    from concourse.bass_utils import run_bass_kernel_spmd

    nc = get_nc()
    in_maps = make_in_maps(x1, x2)
    res = run_bass_kernel_spmd(nc, in_maps, list(range(NCORES)))
    return gather_out(res.results)



# revision 7
# speedup vs baseline: 1.0625x; 1.0625x over previous
"""Euclidean distance block (retrieval kNN) on 8 TRN2 NeuronCores.

dist[b, s, p] = sqrt(sum_c (x1[b, c, p] - x2[b, s, c, p])^2)   p = spatial (h*w)
out[b] = dist[b].reshape(S * h * w)

Sharding: data-parallel over batch B=32 -> 4 batches per core, no comms.

Per-core layout (spatial-split): SBUF partitions carry (channel, spatial_half)
= 64*2 = 128; the free axis carries the 882 spatial positions of one half.
Every tensor reshapes cleanly onto 128 partitions:
  x1[b]  -> [128, 882]      one DMA per batch, no partition-duplicate copy
  x2[b,s]-> [128, 882]      streamed as [128, 2, 882] two-support tiles
  out[b] -> rows (s, si)    [50, 882] f32, contiguous 3528 B per row

Compute chain per support: DVE subtract in bf16 (2x mode, in place), Square
split between ACT and DVE (cadence balance), then one [128, 50] one-hot mask
matmul per spatial half accumulating sum-over-C into PSUM [50, 441] (rows =
(s, si)); supports accumulate start/stop over s = 0..24.  LDWEIGHTS streams on
PE's second read port so per-support mask reloads pipeline behind the matmuls.
ACT Sqrt PSUM -> SBUF f32, one 176 KB store per batch on the otherwise-idle
Sync HWDGE ring (loads-with-cast must use the GpSimd SWDGE ring, and a store
queued behind loads on the same ring would stall them).
"""

import numpy as np

B, S, C, H, W = 32, 25, 64, 42, 42
HW = H * W            # 1764
PHALF = HW // 2       # 882
QW = PHALF // 2       # 441
NCORES = 8
BL = B // NCORES      # 4 batches per core
NSO = 2               # supports per streamed tile
NTILE = S // NSO      # 12 two-support tiles, then 1 leftover single

_cache = {}


def _build_nc():
    import concourse.bacc as bacc
    import concourse.mybir as mybir
    from concourse.tile import TileContext
    from concourse.bass import MemorySpace

    f32 = mybir.dt.float32
    bf16 = mybir.dt.bfloat16
    Square = mybir.ActivationFunctionType.Square
    Sqrt = mybir.ActivationFunctionType.Sqrt
    sub = mybir.AluOpType.subtract
    mult = mybir.AluOpType.mult

    # Square and Sqrt both live in the "sqrt_and_others" act-function set,
    # but the table-load chooser picks the first set containing each one,
    # alternating two ~2.7us table reloads per batch. Strip the two
    # functions from every other set (contents only — set ids are
    # positional) so one resident table serves the whole kernel.
    _orig_tables = bacc.get_activation_tables

    def _pinned_tables(arch):
        t = _orig_tables(arch)
        for name, fns in t.items():
            if name != "sqrt_and_others":
                fns.discard(Square)
                fns.discard(Sqrt)
        return t

    bacc.get_activation_tables = _pinned_tables
    nc = bacc.Bacc()
    x1 = nc.declare_dram_parameter("x1", [BL, C, HW], f32, isOutput=False)
    x2 = nc.declare_dram_parameter("x2", [BL, S, C, HW], f32, isOutput=False)
    mk = nc.declare_dram_parameter("mask", [128, S, 2 * S], bf16, isOutput=False)
    out = nc.declare_dram_parameter("out", [BL, S * HW], f32, isOutput=True)

    with TileContext(nc) as tc:
        with (
            tc.tile_pool(name="x2p", bufs=12) as x2p,
            tc.tile_pool(name="sqp", bufs=4) as sqp,
            tc.tile_pool(name="x1p", bufs=2) as x1p,
            tc.tile_pool(name="outp", bufs=2) as outp,
            tc.tile_pool(name="cst", bufs=1) as cst,
            tc.tile_pool(name="ps", bufs=3, space=MemorySpace.PSUM) as psp,
        ):
            # mask rows are (s, si): mt[k, s, 2s+si(k)] = 1; contiguous
            # 2500 B per partition, one clean HWDGE load
            mt = cst.tile([128, S, 2 * S], bf16)
            nc.sync.dma_start(mt[:], mk[:, :, :])

            for b in range(BL):
                # x1[b]: partition (si, c), one contiguous 3528 B run per
                # partition, cast-loaded just ahead of its batch's stream
                x1bt = x1p.tile([128, PHALF], bf16, tag="x1b")
                nc.gpsimd.dma_start(
                    x1bt[:], x1[b].rearrange("c (si p) -> (c si) p", si=2)
                )
                # stream all supports of the batch: 12 two-support tiles
                # plus the odd support 24 as a single-support tile
                tiles = []
                for i in range(NTILE):
                    x2t = x2p.tile([128, NSO, PHALF], bf16, tag="x2t")
                    nc.gpsimd.dma_start(
                        x2t[:],
                        x2[b, NSO * i : NSO * (i + 1)].rearrange(
                            "s c (si p) -> (c si) s p", si=2
                        ),
                    )
                    tiles.append(x2t)
                x2l = x2p.tile([128, 1, PHALF], bf16, tag="x2l")
                nc.gpsimd.dma_start(
                    x2l[:], x2[b, S - 1 :].rearrange("s c (si p) -> (c si) s p", si=2)
                )

                x1b = x1bt[:, :]
                pst = [
                    psp.tile([2 * S, QW], f32, name=f"ps{h}", tag=f"ps{h}")
                    for h in range(2)
                ]

                for s in range(S):
                    if s < S - 1:
                        d = tiles[s // NSO][:, s % NSO, :]
                    else:
                        d = x2l[:, 0, :]
                    nc.vector.tensor_tensor(d, d, x1b, sub)
                    sq = sqp.tile([128, PHALF], bf16, tag="sq")
                    # squares split 2/1 ACT/DVE to balance engine cadence
                    if s % 3 != 2:
                        nc.scalar.activation(sq[:], d, Square)
                    else:
                        nc.vector.tensor_tensor(sq[:], d, d, mult)
                    for h in range(2):
                        nc.tensor.matmul(
                            pst[h][:, :],
                            mt[:, s, :],
                            sq[:, h * QW : (h + 1) * QW],
                            start=(s == 0),
                            stop=(s == S - 1),
                        )

                # rows (s, si) of out[b] are contiguous 3528 B: store the
                # whole batch in one DMA on the idle Sync HWDGE ring
                ot = outp.tile([2 * S, PHALF], f32, name="ot", tag="ot")
                for h in range(2):
                    nc.scalar.activation(
                        ot[:, h * QW : (h + 1) * QW], pst[h][:], Sqrt
                    )
                nc.sync.dma_start(
                    out[b].rearrange("(s si p) -> (s si) p", si=2, p=PHALF), ot[:]
                )

    try:
        nc.finalize()
    finally:
        bacc.get_activation_tables = _orig_tables
    return nc


def get_nc():
    if "nc" not in _cache:
        _cache["nc"] = _build_nc()
    return _cache["nc"]


def make_mask() -> np.ndarray:
    # mask[k, s, m] = 1 iff the sum of partition k (= channel c = k//2,
    # spatial half si = k%2) for support s belongs to output row m = 2s + si.
    import ml_dtypes

    mask = np.zeros((128, S, 2 * S), dtype=ml_dtypes.bfloat16)
    for k in range(128):
        si = k % 2
        for s in range(S):
            mask[k, s, 2 * s + si] = 1.0
    return mask


def make_in_maps(x1: np.ndarray, x2: np.ndarray) -> list[dict]:
    x1 = np.ascontiguousarray(np.asarray(x1, dtype=np.float32)).reshape(B, C, HW)
    x2 = np.ascontiguousarray(np.asarray(x2, dtype=np.float32)).reshape(B, S, C, HW)
    mask = make_mask()
    maps = []
    for i in range(NCORES):
        sl = slice(i * BL, (i + 1) * BL)
        maps.append({"x1": x1[sl], "x2": x2[sl], "mask": mask})
    return maps


def gather_out(results: list[dict]) -> np.ndarray:
    return np.concatenate([np.asarray(r["out"]) for r in results], axis=0).astype(
        np.float32, copy=False
    )


def kernel(x1, x2) -> np.ndarray:
    from concourse.bass_utils import run_bass_kernel_spmd

    nc = get_nc()
    in_maps = make_in_maps(x1, x2)
    res = run_bass_kernel_spmd(nc, in_maps, list(range(NCORES)))
    return gather_out(res.results)
